# revision 20
# baseline (speedup 1.0000x reference)
"""YOLO-style DetectionLoss on 8 Trainium2 NeuronCores (Bass/Tile).

Pure data parallelism: batch 8192 -> 1024 per core; 1024*7*7 = 50176
cells laid out as 128 SBUF partitions x 392 cells (each partition owns 8
consecutive images). Per chunk of k cells the kernel builds masked
residual tiles whose squares sum to the loss:

  Vbox[...,b,0:4] = sqrt(5)*resp_b*(dxy | dwh)   (xy + wh terms)
  Vbox[...,b,4]   = resp_b*(pc_b - max_iou)      (contain term)
  Vcls[...,c]     = obj*(pcls_c - tcls_c)        (class term)
  noobj term      = 0.5*noobj*pc_b^2 via Square(pc)+masked accumulate

All loss weights are folded into the masks, so each chunk closes with
two ACT Square+accumulate ops (Vbox, Vcls) plus one Pool masked
accumulate -> 3 accumulator slots per chunk, summed on the host and
divided by the global batch.

The responsible-box one-hot uses reduce_max + is_equal (exact fp match)
instead of pairwise compares; ties can only occur when every IoU in a
cell is exactly 0 (measure-zero effect on the loss).
"""

import os

os.environ.setdefault("JAX_COMPILATION_CACHE_DIR", "/tmp/jaxcache")
os.environ.setdefault("JAX_PERSISTENT_CACHE_MIN_COMPILE_TIME_SECS", "1")
os.environ.setdefault("JAX_PERSISTENT_CACHE_MIN_ENTRY_SIZE_BYTES", "0")

import numpy as np

import concourse.bacc as bacc
import concourse.mybir as mybir
import concourse.tile as tile
from concourse.bass_utils import run_bass_kernel_spmd

F32 = mybir.dt.float32
AF = mybir.ActivationFunctionType
OP = mybir.AluOpType
AX = mybir.AxisListType

NB, C, S = 3, 20, 7
D = 5 * NB + C                 # 35
B = 8192
NCORES = 8
P = 128

SQRT5 = 5.0 ** 0.5
NTERMS = 3                     # box(xy+wh+contain), class, noobj


def default_chunks(kpp):
    if kpp == 392:
        return [98, 147, 147]
    if kpp % 98 == 0:
        return [98] * (kpp // 98)
    if kpp % 49 == 0:
        return [49] * (kpp // 49)
    return [kpp]


def build_nc(bc: int, ks=None, repeats: int = 1, io_bufs: int = 2,
             loop_repeats: int = 0, use_reduce: bool = True,
             use_ttr: bool = False, use_ts2: bool = True,
             cw_pool: bool = False, sq_scale: bool = True):
    """Trace the per-core Bass program for a per-core batch of `bc`."""
    cells = bc * S * S
    assert cells % P == 0
    kpp = cells // P
    if ks is None:
        ks = default_chunks(kpp)
    assert sum(ks) == kpp
    nchunks = len(ks)

    nc = bacc.Bacc("TRN2", debug=False, num_devices=NCORES)
    out_h = nc.dram_tensor("output", [bc, S, S, D], F32, kind="ExternalInput")
    tgt_h = nc.dram_tensor("target", [bc, S, S, D], F32, kind="ExternalInput")
    acc_h = nc.dram_tensor("acc", [P, NTERMS * nchunks], F32,
                           kind="ExternalOutput")

    out_v = out_h.ap().rearrange("(p a) h w d -> p (a h w d)", p=P)
    tgt_v = tgt_h.ap().rearrange("(p a) h w d -> p (a h w d)", p=P)

    with tile.TileContext(nc) as tc:
        with (
            tc.tile_pool(name="io", bufs=io_bufs) as io_pool,
            tc.tile_pool(name="pv", bufs=2) as pv,       # Vbox
            tc.tile_pool(name="pvc", bufs=2) as pvc,     # Vcls
            tc.tile_pool(name="p6", bufs=2) as p6,       # [k,3,2] temps
            tc.tile_pool(name="pw", bufs=2) as pw_pool,  # dwt [k,3,4]
            tc.tile_pool(name="psqrt", bufs=2) as psqrt, # sp/st
            tc.tile_pool(name="p3", bufs=2) as p3,       # [k,3] temps
            tc.tile_pool(name="p1", bufs=2) as p1,       # [k] temps
            tc.tile_pool(name="accp", bufs=1) as accp,
        ):
            acc = accp.tile([P, NTERMS * nchunks], F32)

            import contextlib
            loop_cm = (tc.For_i(0, loop_repeats, 1) if loop_repeats
                       else contextlib.nullcontext())
            with loop_cm:
                for rep in range(repeats):
                    off = 0
                    pending_closings = None
                    for ci, k in enumerate(ks):
                        prev_closings = pending_closings
                        ot = io_pool.tile([P, k * D], F32, name="ot", tag="ot")
                        tt = io_pool.tile([P, k * D], F32, name="tt", tag="tt")
                        nc.sync.dma_start(ot[:], out_v[:, off:off + k * D])
                        nc.sync.dma_start(tt[:], tgt_v[:, off:off + k * D])
                        off += k * D

                        o3 = ot[:].rearrange("p (k d) -> p k d", d=D)
                        t3 = tt[:].rearrange("p (k d) -> p k d", d=D)
                        ob = o3[:, :, 0:15].rearrange("p k (b f) -> p k b f", f=5)
                        tb = t3[:, :, 0:15].rearrange("p k (b f) -> p k b f", f=5)

                        pxy = ob[:, :, :, 0:2]
                        pwh = ob[:, :, :, 2:4]
                        pc_ = ob[:, :, :, 4]
                        twh = tb[:, :, :, 2:4]
                        t0 = tb[:, :, 0, :]
                        tw0 = t3[:, :, 2]
                        th0 = t3[:, :, 3]
                        conf = t3[:, :, 4]
                        ocls = o3[:, :, 15:35]
                        tcls = t3[:, :, 15:35]

                        txy0b = t0[:, :, 0:2].unsqueeze(2).broadcast_to(
                            [P, k, 3, 2])
                        twh0b = t0[:, :, 2:4].unsqueeze(2).broadcast_to(
                            [P, k, 3, 2])
                        conf3 = conf.unsqueeze(2).broadcast_to([P, k, 3])
                        conf20 = conf.unsqueeze(2).broadcast_to([P, k, 20])

                        def slot(term):
                            i = ci * NTERMS + term
                            return acc[:, i:i + 1]

                        # -------- tiles --------
                        V = pv.tile([P, k, 3, 5], F32, name="V", tag="V")[:]
                        Vc = pvc.tile([P, k, 20], F32, name="Vc", tag="Vc")[:]
                        dwt = pw_pool.tile([P, k, 3, 4], F32, name="dwt",
                                           tag="dwt")[:]
                        sp = psqrt.tile([P, k, 3, 2], F32, name="sp", tag="sp")[:]
                        st = psqrt.tile([P, k, 3, 2], F32, name="st", tag="st")[:]
                        dcx = p6.tile([P, k, 3, 2], F32, name="dcx", tag="dcx")[:]
                        spt = p6.tile([P, k, 3, 2], F32, name="spt", tag="spt")[:]
                        m = p6.tile([P, k, 3, 2], F32, name="m", tag="m")[:]
                        inter = p3.tile([P, k, 3], F32, name="inter", tag="inter")[:]
                        a1 = p3.tile([P, k, 3], F32, name="a1", tag="a1")[:]
                        s4 = p3.tile([P, k, 3], F32, name="s4", tag="s4")[:]
                        a24 = p1.tile([P, k], F32, name="a24", tag="a24")[:]
                        rcp = p3.tile([P, k, 3], F32, name="rcp", tag="rcp")[:]
                        miou = p1.tile([P, k], F32, name="miou", tag="miou")[:]
                        e = p3.tile([P, k, 3], F32, name="e", tag="e")[:]
                        rm5 = p3.tile([P, k, 3], F32, name="rm5", tag="rm5")[:]
                        dc = p3.tile([P, k, 3], F32, name="dc", tag="dc")[:]
                        pcm = p3.tile([P, k, 3], F32, name="pcm", tag="pcm")[:]
                        nm = p1.tile([P, k], F32, name="nm", tag="nm")[:]

                        a24b = a24.unsqueeze(2).broadcast_to([P, k, 3])
                        mioub = miou.unsqueeze(2).broadcast_to([P, k, 3])
                        nm3b = nm.unsqueeze(2).broadcast_to([P, k, 3])
                        rm5b4 = rm5.unsqueeze(3).broadcast_to([P, k, 3, 4])

                        # -------- ACT: early unary work --------
                        nc.scalar.activation(sp, pwh, AF.Sqrt)
                        nc.scalar.activation(st, twh, AF.Sqrt)

                        # -------- DVE: nm first (Pool pcm needs it) --------
                        # nm = 0.5 * (conf != 1)  (noobj weight folded in)
                        nc.vector.tensor_scalar(nm, conf, 1.0, 0.5,
                                                op0=OP.not_equal, op1=OP.mult)

                        # -------- Pool: no in-place writes (drain-free) ----
                        nc.gpsimd.tensor_sub(dwt[:, :, :, 0:2], pxy,
                                             tb[:, :, :, 0:2])
                        nc.gpsimd.tensor_mul(a1, ob[:, :, :, 2], ob[:, :, :, 3])
                        nc.gpsimd.tensor_mul(a24, tw0, th0)
                        nc.gpsimd.tensor_add(s4, a1, a24b)
                        nc.gpsimd.tensor_sub(dwt[:, :, :, 2:4], sp, st)
                        nc.gpsimd.tensor_sub(Vc, ocls, tcls)
                        # pcm = nm * pc  ((sqrt2*pcm)^2 = 0.5*noobj*pc^2)
                        nc.gpsimd.tensor_mul(pcm, pc_, nm3b)

                        # -------- DVE: IoU / responsibility chain --------
                        nc.vector.tensor_sub(dcx, pxy, txy0b)
                        nc.scalar.activation(dcx, dcx, AF.Abs, scale=2.0 / S)
                        nc.vector.tensor_add(spt, pwh, twh0b)
                        nc.vector.tensor_sub(spt, spt, dcx)      # u, in place
                        nc.vector.tensor_tensor(m, pwh, twh0b, op=OP.min)
                        nc.vector.scalar_tensor_tensor(
                            m, m, 2.0, spt, op0=OP.mult, op1=OP.min)
                        nc.scalar.activation(m, m, AF.Relu)
                        if prev_closings is not None:
                            prev_closings()
                        nc.vector.tensor_mul(inter, m[:, :, :, 0], m[:, :, :, 1])
                        nc.vector.scalar_tensor_tensor(          # den4, in place
                            s4, s4, 4.0, inter, op0=OP.mult, op1=OP.subtract)
                        nc.vector.reciprocal(rcp, s4)
                        nc.vector.tensor_mul(inter, inter, rcp)  # iou, in place
                        if use_reduce:
                            nc.vector.tensor_reduce(miou, inter, axis=AX.X,
                                                    op=OP.max)
                            nc.vector.tensor_tensor(e, inter, mioub,
                                                    op=OP.is_equal)
                            nc.vector.tensor_mul(e, e, conf3)    # resp, in place
                        else:
                            i0, i1, i2 = (inter[:, :, 0], inter[:, :, 1],
                                          inter[:, :, 2])
                            c01 = p1.tile([P, k], F32, name="c01", tag="c01")[:]
                            nc.vector.tensor_tensor(c01, i0, i1, op=OP.is_ge)
                            c02 = p1.tile([P, k], F32, name="c02", tag="c02")[:]
                            nc.vector.tensor_tensor(c02, i0, i2, op=OP.is_ge)
                            c12 = p1.tile([P, k], F32, name="c12", tag="c12")[:]
                            nc.vector.tensor_tensor(c12, i1, i2, op=OP.is_ge)
                            r0, r1, r2 = e[:, :, 0], e[:, :, 1], e[:, :, 2]
                            nc.vector.tensor_mul(r0, c01, c02)
                            nc.vector.scalar_tensor_tensor(
                                r1, r0, 1.0, c12, op0=OP.not_equal, op1=OP.mult)
                            t01 = p1.tile([P, k], F32, name="t01", tag="t01")[:]
                            nc.vector.tensor_add(t01, r0, r1)
                            nc.vector.tensor_scalar(
                                r2, t01, 1.0, None, op0=OP.not_equal)
                            nc.vector.tensor_mul(e, e, conf3)
                        nc.vector.tensor_scalar_mul(rm5, e, SQRT5)
                        if use_reduce:
                            nc.vector.tensor_sub(dc, pc_, mioub)
                        else:
                            nc.vector.tensor_sub(dc, pc_, inter)
                        nc.vector.tensor_mul(V[:, :, :, 4], dc, e)
                        # class mask (in place on DVE; Pool wrote the sub)
                        nc.vector.tensor_mul(Vc, Vc, conf20)

                        # -------- masked box residuals --------
                        if cw_pool:
                            nc.gpsimd.tensor_mul(V[:, :, :, 0:4], dwt, rm5b4)
                        else:
                            nc.vector.tensor_mul(V[:, :, :, 0:4], dwt, rm5b4)

                        # -------- ACT closings: deferred to next chunk --------
                        def make_closings(pcm=pcm, V=V, Vc=Vc, ci=ci):
                            def emit():
                                nc.scalar.activation(
                                    pcm, pcm, AF.Square, scale=2.0 ** 0.5,
                                    accum_out=acc[:, ci * NTERMS + 2:
                                                  ci * NTERMS + 3])
                                nc.scalar.activation(
                                    V, V, AF.Square,
                                    accum_out=acc[:, ci * NTERMS:
                                                  ci * NTERMS + 1])
                                nc.scalar.activation(
                                    Vc, Vc, AF.Square,
                                    accum_out=acc[:, ci * NTERMS + 1:
                                                  ci * NTERMS + 2])
                            return emit
                        pending_closings = make_closings()

                    if pending_closings is not None:
                        pending_closings()
                        pending_closings = None

            nc.sync.dma_start(acc_h.ap()[:], acc[:])

    nc.compile()
    return nc


_CACHE = {}


def _get_nc(bc, ks=None, repeats=1, io_bufs=2, loop_repeats=0):
    key = (bc, tuple(ks) if ks else None, repeats, io_bufs, loop_repeats)
    if key not in _CACHE:
        _CACHE[key] = build_nc(bc, ks, repeats, io_bufs, loop_repeats)
    return _CACHE[key]


def combine_acc(acc_list, nchunks):
    tot = 0.0
    for a in acc_list:
        tot += a.astype(np.float64).sum()
    return np.float32(tot / B)


BEST_KS = [98, 147, 147]
BEST_IO_BUFS = 2


def kernel(output: np.ndarray, target: np.ndarray) -> np.ndarray:
    assert output.shape == (B, S, S, D) and target.shape == (B, S, S, D)
    bc = B // NCORES
    nchunks = len(BEST_KS)
    nc = _get_nc(bc, BEST_KS, io_bufs=BEST_IO_BUFS)
    in_maps = [
        {
            "output": np.ascontiguousarray(output[i * bc:(i + 1) * bc]),
            "target": np.ascontiguousarray(target[i * bc:(i + 1) * bc]),
        }
        for i in range(NCORES)
    ]
    res = run_bass_kernel_spmd(nc, in_maps, list(range(NCORES)))
    return combine_acc([r["acc"] for r in res.results], nchunks)


# revision 21
# speedup vs baseline: 1.0414x; 1.0414x over previous
"""YOLO-style DetectionLoss on 8 Trainium2 NeuronCores (Bass/Tile).

Pure data parallelism: batch 8192 -> 1024 per core; 1024*7*7 = 50176
cells laid out as 128 SBUF partitions x 392 cells (each partition owns 8
consecutive images). Per chunk of k cells the kernel builds masked
residual tiles whose squares sum to the loss:

  Vbox[...,b,0:4] = sqrt(5)*resp_b*(dxy | dwh)   (xy + wh terms)
  Vbox[...,b,4]   = resp_b*(pc_b - max_iou)      (contain term)
  Vcls[...,c]     = obj*(pcls_c - tcls_c)        (class term)
  noobj term      = 0.5*noobj*pc_b^2 via Square(pc)+masked accumulate

All loss weights are folded into the masks, so each chunk closes with
two ACT Square+accumulate ops (Vbox, Vcls) plus one Pool masked
accumulate -> 3 accumulator slots per chunk, summed on the host and
divided by the global batch.

The responsible-box one-hot uses reduce_max + is_equal (exact fp match)
instead of pairwise compares; ties can only occur when every IoU in a
cell is exactly 0 (measure-zero effect on the loss).
"""

import os

os.environ.setdefault("JAX_COMPILATION_CACHE_DIR", "/tmp/jaxcache")
os.environ.setdefault("JAX_PERSISTENT_CACHE_MIN_COMPILE_TIME_SECS", "1")
os.environ.setdefault("JAX_PERSISTENT_CACHE_MIN_ENTRY_SIZE_BYTES", "0")

import numpy as np

import concourse.bacc as bacc
import concourse.mybir as mybir
import concourse.tile as tile
from concourse.bass_utils import run_bass_kernel_spmd

F32 = mybir.dt.float32
AF = mybir.ActivationFunctionType
OP = mybir.AluOpType
AX = mybir.AxisListType

NB, C, S = 3, 20, 7
D = 5 * NB + C                 # 35
B = 8192
NCORES = 8
P = 128

SQRT5 = 5.0 ** 0.5
NTERMS = 3                     # box(xy+wh+contain), class, noobj


def default_chunks(kpp):
    if kpp == 392:
        return [98, 147, 147]
    if kpp % 98 == 0:
        return [98] * (kpp // 98)
    if kpp % 49 == 0:
        return [49] * (kpp // 49)
    return [kpp]


def build_nc(bc: int, ks=None, repeats: int = 1, io_bufs: int = 2,
             loop_repeats: int = 0, use_reduce: bool = True,
             use_ttr: bool = False, use_ts2: bool = True,
             cw_pool: bool = False, sq_scale: bool = True):
    """Trace the per-core Bass program for a per-core batch of `bc`."""
    cells = bc * S * S
    assert cells % P == 0
    kpp = cells // P
    if ks is None:
        ks = default_chunks(kpp)
    assert sum(ks) == kpp
    nchunks = len(ks)

    nc = bacc.Bacc("TRN2", debug=False, num_devices=NCORES)
    out_h = nc.dram_tensor("output", [bc, S, S, D], F32, kind="ExternalInput")
    tgt_h = nc.dram_tensor("target", [bc, S, S, D], F32, kind="ExternalInput")
    acc_h = nc.dram_tensor("acc", [P, NTERMS * nchunks], F32,
                           kind="ExternalOutput")

    out_v = out_h.ap().rearrange("(p a) h w d -> p (a h w d)", p=P)
    tgt_v = tgt_h.ap().rearrange("(p a) h w d -> p (a h w d)", p=P)

    with tile.TileContext(nc) as tc:
        with (
            tc.tile_pool(name="io", bufs=io_bufs) as io_pool,
            tc.tile_pool(name="pv", bufs=2) as pv,       # Vbox
            tc.tile_pool(name="pvc", bufs=2) as pvc,     # Vcls
            tc.tile_pool(name="p6", bufs=2) as p6,       # [k,3,2] temps
            tc.tile_pool(name="pw", bufs=2) as pw_pool,  # dwt [k,3,4]
            tc.tile_pool(name="psqrt", bufs=2) as psqrt, # sp/st
            tc.tile_pool(name="p3", bufs=2) as p3,       # [k,3] temps
            tc.tile_pool(name="p1", bufs=2) as p1,       # [k] temps
            tc.tile_pool(name="accp", bufs=1) as accp,
        ):
            acc = accp.tile([P, NTERMS * nchunks], F32)

            import contextlib
            loop_cm = (tc.For_i(0, loop_repeats, 1) if loop_repeats
                       else contextlib.nullcontext())
            with loop_cm:
                for rep in range(repeats):
                    off = 0
                    pending_closings = None
                    for ci, k in enumerate(ks):
                        prev_closings = pending_closings
                        ot = io_pool.tile([P, k * D], F32, name="ot", tag="ot")
                        tt = io_pool.tile([P, k * D], F32, name="tt", tag="tt")
                        nc.sync.dma_start(ot[:], out_v[:, off:off + k * D])
                        nc.sync.dma_start(tt[:], tgt_v[:, off:off + k * D])
                        off += k * D

                        o3 = ot[:].rearrange("p (k d) -> p k d", d=D)
                        t3 = tt[:].rearrange("p (k d) -> p k d", d=D)
                        ob = o3[:, :, 0:15].rearrange("p k (b f) -> p k b f", f=5)
                        tb = t3[:, :, 0:15].rearrange("p k (b f) -> p k b f", f=5)

                        pxy = ob[:, :, :, 0:2]
                        pwh = ob[:, :, :, 2:4]
                        pc_ = ob[:, :, :, 4]
                        twh = tb[:, :, :, 2:4]
                        t0 = tb[:, :, 0, :]
                        tw0 = t3[:, :, 2]
                        th0 = t3[:, :, 3]
                        conf = t3[:, :, 4]
                        ocls = o3[:, :, 15:35]
                        tcls = t3[:, :, 15:35]

                        txy0b = t0[:, :, 0:2].unsqueeze(2).broadcast_to(
                            [P, k, 3, 2])
                        twh0b = t0[:, :, 2:4].unsqueeze(2).broadcast_to(
                            [P, k, 3, 2])
                        conf3 = conf.unsqueeze(2).broadcast_to([P, k, 3])
                        conf20 = conf.unsqueeze(2).broadcast_to([P, k, 20])

                        def slot(term):
                            i = ci * NTERMS + term
                            return acc[:, i:i + 1]

                        # -------- tiles --------
                        V = pv.tile([P, k, 3, 5], F32, name="V", tag="V")[:]
                        Vc = pvc.tile([P, k, 20], F32, name="Vc", tag="Vc")[:]
                        dwt = pw_pool.tile([P, k, 3, 4], F32, name="dwt",
                                           tag="dwt")[:]
                        sp = psqrt.tile([P, k, 3, 2], F32, name="sp", tag="sp")[:]
                        st = psqrt.tile([P, k, 3, 2], F32, name="st", tag="st")[:]
                        dcx = p6.tile([P, k, 3, 2], F32, name="dcx", tag="dcx")[:]
                        spt = p6.tile([P, k, 3, 2], F32, name="spt", tag="spt")[:]
                        m = p6.tile([P, k, 3, 2], F32, name="m", tag="m")[:]
                        inter = p3.tile([P, k, 3], F32, name="inter", tag="inter")[:]
                        a1 = p3.tile([P, k, 3], F32, name="a1", tag="a1")[:]
                        s4 = p3.tile([P, k, 3], F32, name="s4", tag="s4")[:]
                        a24 = p1.tile([P, k], F32, name="a24", tag="a24")[:]
                        rcp = p3.tile([P, k, 3], F32, name="rcp", tag="rcp")[:]
                        miou = p1.tile([P, k], F32, name="miou", tag="miou")[:]
                        e = p3.tile([P, k, 3], F32, name="e", tag="e")[:]
                        rm5 = p3.tile([P, k, 3], F32, name="rm5", tag="rm5")[:]
                        dc = p3.tile([P, k, 3], F32, name="dc", tag="dc")[:]
                        pcm = p3.tile([P, k, 3], F32, name="pcm", tag="pcm")[:]
                        nm = p1.tile([P, k], F32, name="nm", tag="nm")[:]

                        a24b = a24.unsqueeze(2).broadcast_to([P, k, 3])
                        mioub = miou.unsqueeze(2).broadcast_to([P, k, 3])
                        nm3b = nm.unsqueeze(2).broadcast_to([P, k, 3])
                        rm5b4 = rm5.unsqueeze(3).broadcast_to([P, k, 3, 4])

                        # -------- ACT: early unary work --------
                        nc.scalar.activation(sp, pwh, AF.Sqrt)
                        nc.scalar.activation(st, twh, AF.Sqrt)

                        # -------- DVE: nm first (Pool pcm needs it) --------
                        # nm = 0.5 * (conf != 1)  (noobj weight folded in)
                        nc.vector.tensor_scalar(nm, conf, 1.0, 0.5,
                                                op0=OP.not_equal, op1=OP.mult)

                        # -------- Pool: no in-place writes (drain-free) ----
                        nc.gpsimd.tensor_sub(dwt[:, :, :, 0:2], pxy,
                                             tb[:, :, :, 0:2])
                        nc.gpsimd.tensor_mul(a1, ob[:, :, :, 2], ob[:, :, :, 3])
                        nc.gpsimd.tensor_mul(a24, tw0, th0)
                        nc.gpsimd.tensor_add(s4, a1, a24b)
                        nc.gpsimd.tensor_sub(dwt[:, :, :, 2:4], sp, st)
                        nc.gpsimd.tensor_sub(Vc, ocls, tcls)
                        # pcm = nm * pc  ((sqrt2*pcm)^2 = 0.5*noobj*pc^2)
                        nc.gpsimd.tensor_mul(pcm, pc_, nm3b)

                        # -------- DVE: IoU / responsibility chain --------
                        nc.vector.tensor_sub(dcx, pxy, txy0b)
                        nc.scalar.activation(dcx, dcx, AF.Abs, scale=2.0 / S)
                        nc.vector.tensor_add(spt, pwh, twh0b)
                        nc.vector.tensor_sub(spt, spt, dcx)      # u, in place
                        nc.vector.tensor_tensor(m, pwh, twh0b, op=OP.min)
                        nc.vector.scalar_tensor_tensor(
                            m, m, 2.0, spt, op0=OP.mult, op1=OP.min)
                        nc.scalar.activation(m, m, AF.Relu)
                        if prev_closings is not None:
                            prev_closings()
                        nc.vector.tensor_mul(inter, m[:, :, :, 0], m[:, :, :, 1])
                        nc.vector.scalar_tensor_tensor(          # den4, in place
                            s4, s4, 4.0, inter, op0=OP.mult, op1=OP.subtract)
                        nc.vector.reciprocal(rcp, s4)
                        nc.vector.tensor_mul(inter, inter, rcp)  # iou, in place
                        if use_reduce:
                            nc.vector.tensor_reduce(miou, inter, axis=AX.X,
                                                    op=OP.max)
                            nc.vector.tensor_tensor(e, inter, mioub,
                                                    op=OP.is_equal)
                            nc.vector.tensor_mul(e, e, conf3)    # resp, in place
                        else:
                            i0, i1, i2 = (inter[:, :, 0], inter[:, :, 1],
                                          inter[:, :, 2])
                            c01 = p1.tile([P, k], F32, name="c01", tag="c01")[:]
                            nc.vector.tensor_tensor(c01, i0, i1, op=OP.is_ge)
                            c02 = p1.tile([P, k], F32, name="c02", tag="c02")[:]
                            nc.vector.tensor_tensor(c02, i0, i2, op=OP.is_ge)
                            c12 = p1.tile([P, k], F32, name="c12", tag="c12")[:]
                            nc.vector.tensor_tensor(c12, i1, i2, op=OP.is_ge)
                            r0, r1, r2 = e[:, :, 0], e[:, :, 1], e[:, :, 2]
                            nc.vector.tensor_mul(r0, c01, c02)
                            nc.vector.scalar_tensor_tensor(
                                r1, r0, 1.0, c12, op0=OP.not_equal, op1=OP.mult)
                            t01 = p1.tile([P, k], F32, name="t01", tag="t01")[:]
                            nc.vector.tensor_add(t01, r0, r1)
                            nc.vector.tensor_scalar(
                                r2, t01, 1.0, None, op0=OP.not_equal)
                            nc.vector.tensor_mul(e, e, conf3)
                        nc.vector.tensor_scalar_mul(rm5, e, SQRT5)
                        if use_reduce:
                            nc.vector.tensor_sub(dc, pc_, mioub)
                        else:
                            nc.vector.tensor_sub(dc, pc_, inter)
                        nc.vector.tensor_mul(V[:, :, :, 4], dc, e)
                        # class mask (in place on DVE; Pool wrote the sub)
                        nc.vector.tensor_mul(Vc, Vc, conf20)

                        # -------- masked box residuals --------
                        if cw_pool:
                            nc.gpsimd.tensor_mul(V[:, :, :, 0:4], dwt, rm5b4)
                        else:
                            nc.vector.tensor_mul(V[:, :, :, 0:4], dwt, rm5b4)

                        # -------- ACT closings: deferred to next chunk --------
                        def make_closings(pcm=pcm, V=V, Vc=Vc, ci=ci):
                            def emit():
                                nc.scalar.activation(
                                    pcm, pcm, AF.Square, scale=2.0 ** 0.5,
                                    accum_out=acc[:, ci * NTERMS + 2:
                                                  ci * NTERMS + 3])
                                nc.scalar.activation(
                                    V, V, AF.Square,
                                    accum_out=acc[:, ci * NTERMS:
                                                  ci * NTERMS + 1])
                                nc.scalar.activation(
                                    Vc, Vc, AF.Square,
                                    accum_out=acc[:, ci * NTERMS + 1:
                                                  ci * NTERMS + 2])
                            return emit
                        pending_closings = make_closings()

                    if pending_closings is not None:
                        pending_closings()
                        pending_closings = None

            nc.sync.dma_start(acc_h.ap()[:], acc[:])

    nc.compile()
    return nc


_CACHE = {}


def _get_nc(bc, ks=None, repeats=1, io_bufs=2, loop_repeats=0):
    key = (bc, tuple(ks) if ks else None, repeats, io_bufs, loop_repeats)
    if key not in _CACHE:
        _CACHE[key] = build_nc(bc, ks, repeats, io_bufs, loop_repeats)
    return _CACHE[key]


def combine_acc(acc_list, nchunks):
    tot = 0.0
    for a in acc_list:
        tot += a.astype(np.float64).sum()
    return np.float32(tot / B)


BEST_KS = [28, 49, 49, 56, 56, 56, 49, 49]
BEST_IO_BUFS = 6


def kernel(output: np.ndarray, target: np.ndarray) -> np.ndarray:
    assert output.shape == (B, S, S, D) and target.shape == (B, S, S, D)
    bc = B // NCORES
    nchunks = len(BEST_KS)
    nc = _get_nc(bc, BEST_KS, io_bufs=BEST_IO_BUFS)
    in_maps = [
        {
            "output": np.ascontiguousarray(output[i * bc:(i + 1) * bc]),
            "target": np.ascontiguousarray(target[i * bc:(i + 1) * bc]),
        }
        for i in range(NCORES)
    ]
    res = run_bass_kernel_spmd(nc, in_maps, list(range(NCORES)))
    return combine_acc([r["acc"] for r in res.results], nchunks)
